# revision 2
# baseline (speedup 1.0000x reference)
"""Trainium2 Bass kernel for nn_CBSA_45389214384209 (sparse_attention).

Reference computation (per batch element b of 8):
  x_seq = x[b].T                      # [4096, 256]   (x[b] is [256, 4096])
  proj  = x_seq @ W_proj              # [4096, 512]
  rep   = avgpool8x8(proj)            # [64, 512]  == avgpool8x8(x_seq) @ W_proj
  per head h (8 heads, dh=64):
    S    = rep_h @ proj_h.T * scale   # [64, 4096]
    P    = softmax(S)                 # [64, 4096]
    rd   = P @ proj_h                 # [64, 64]
    rep2 = rep_h + step_rep[h] * rd
    P2   = softmax(rep2 @ rep2.T * scale)
    xd2  = step_x[h] * (P2 @ rep2)    # [64, 64]
    xdT  = xd2.T @ P                  # [64, 4096]  (back-projection, transposed)
  out[b] = W_out.T @ concat_h(xdT) + b_out[:, None]   # [256, 4096]

Sharding: pure data parallel - one batch element per NeuronCore (8 cores).

Layout strategy: everything is computed "transposed" (feature dim on
partitions, token dim on the free axis) so the kernel needs no edge
transposes: projT = W_proj.T @ x[b] comes straight from the native DRAM
layout of x, and the final x_outT[c, t] is exactly the DRAM layout of the
output.  Head pairs are packed into disjoint 64x64 PE quadrants via
tile_position.  Large matmuls with fp32 operands use float32r (1 cyc/row);
attention score/apply stages run in bf16 (scores are ~N(0, 1/64) here, so
bf16 rounding is far below the fp32 envelope); softmax is exact.
"""

import os
import sys

import numpy as np

for _p in ("/opt/trn_rl_repo", os.path.expanduser("~/.axon_site/_ro/trn_rl_repo")):
    if os.path.isdir(_p) and _p not in sys.path:
        sys.path.insert(0, _p)

import concourse.bass as bass
import concourse.tile as tile
from concourse import bacc, mybir
from concourse.bass import ds, ts
from concourse.masks import make_identity

F32 = mybir.dt.float32
F32R = mybir.dt.float32r
BF16 = mybir.dt.bfloat16
AX = mybir.AxisListType
ALU = mybir.AluOpType
ACTF = mybir.ActivationFunctionType

B = 8
C = 256          # model dim
T = 4096         # tokens (64x64 grid)
INNER = 512
HEADS = 8
DH = 64
NB = 64          # pooled tokens (8x8 grid)
SCALE = DH ** -0.5
NPAIR = 4        # head pairs
NCHUNK = 8       # 512-wide token chunks
NTT = 32         # 128-wide token tiles

CFG = {"p_mode": "bf16"}


def build_module(cfg=CFG):
    nc = bacc.Bacc("TRN2", debug=False)

    x = nc.dram_tensor("x", [C, T], F32, kind="ExternalInput").ap()
    wp = nc.dram_tensor("w_proj", [C, INNER], F32, kind="ExternalInput").ap()
    wo = nc.dram_tensor("w_out", [INNER, C], F32, kind="ExternalInput").ap()
    bo = nc.dram_tensor("b_out", [C], F32, kind="ExternalInput").ap()
    srep = nc.dram_tensor("s_rep", [HEADS], F32, kind="ExternalInput").ap()
    sx = nc.dram_tensor("s_x", [HEADS], F32, kind="ExternalInput").ap()
    out = nc.dram_tensor("out", [C, T], F32, kind="ExternalOutput").ap()

    with tile.TileContext(nc) as tc:
        _body(tc, cfg, x, wp, wo, bo, srep, sx, out)
    nc.compile()
    return nc


def _body(tc, cfg, x, wp, wo, bo, srep, sx, out):
    nc = tc.nc

    x_r = x.rearrange("(o p) t -> p o t", p=128)      # [128, 2, 4096]
    out_r = out.rearrange("(o p) t -> p o t", p=128)  # [128, 2, 4096]

    # ---- pools (SBUF pools stack-nested: alloc order == reverse release) --
    consts = tc.alloc_tile_pool(name="consts", bufs=1)
    stats = tc.alloc_tile_pool(name="stats", bufs=2)
    outp = tc.alloc_tile_pool(name="outp", bufs=3)       # out staging
    xdtp = tc.alloc_tile_pool(name="xdtp", bufs=2)       # xdT chunks
    pp = tc.alloc_tile_pool(name="pp", bufs=1)           # P (attn) tiles
    xd2p = tc.alloc_tile_pool(name="xd2p", bufs=1)       # stage-2 outputs
    prp = tc.alloc_tile_pool(name="prp", bufs=1)         # proj (bf16)
    b3 = tc.alloc_tile_pool(name="b3", bufs=2)           # stage-2 temps
    b2 = tc.alloc_tile_pool(name="b2", bufs=3)           # P^T chunks
    pTp = tc.alloc_tile_pool(name="pTp", bufs=1)         # projT (bf16)
    xp = tc.alloc_tile_pool(name="xp", bufs=1)           # x streaming tiles

    # One PSUM pool for the whole kernel; tags shared across phases.
    # mm(4) + acc64(2) + b3s(2) = 8 banks (dma mode); pe mode: mm=3 tp=1.
    psum = tc.alloc_tile_pool(name="psum", bufs=1, space="PSUM")
    mm_bufs = 3

    # ---- constants / weights -------------------------------------------
    ident_bf = consts.tile([128, 128], BF16, name="ident_bf")
    make_identity(nc, ident_bf)
    ident_f = consts.tile([128, 128], F32, name="ident_f")
    make_identity(nc, ident_f)

    # fp32r matmul operands must be produced (rounded) by a compute op,
    # so weights get a one-time rounding copy after the DMA load.
    wp_sb = consts.tile([128, 2, INNER], F32, name="wp_sb")
    nc.sync.dma_start(wp_sb, wp.rearrange("(o p) i -> p o i", p=128))
    wp_r = consts.tile([128, 2, INNER], F32R, name="wp_r")
    nc.vector.tensor_copy(wp_r, wp_sb)
    wo_sb = consts.tile([128, 4, C], F32, name="wo_sb")
    nc.sync.dma_start(wo_sb, wo.rearrange("(g p) c -> p g c", p=128))
    wo_r = consts.tile([128, 4, C], F32R, name="wo_r")
    nc.vector.tensor_copy(wo_r, wo_sb)
    bo_ld = consts.tile([128, 2], F32, name="bo_ld")
    nc.sync.dma_start(bo_ld, bo.rearrange("(o p) -> p o", p=128))
    bo_sb = consts.tile([128, 2], F32, name="bo_sb")
    nc.vector.tensor_copy(bo_sb, bo_ld)

    # step_rep / step_x broadcast per pair: column p holds step[2p] on
    # partitions 0-63 and step[2p+1] on partitions 64-127.
    ones_row = consts.tile([1, 128], F32, name="ones_row")
    nc.vector.memset(ones_row, 1.0)
    # DMA-loaded constants are re-staged through a DVE copy so downstream
    # DVE consumers are same-engine (keeps per-instruction wait counts <= 1).
    srep_ld = consts.tile([128, HEADS], F32, name="srep_ld")
    sx_ld = consts.tile([128, HEADS], F32, name="sx_ld")
    srep_bc = consts.tile([128, NPAIR], F32, name="srep_bc")
    sx_bc = consts.tile([128, NPAIR], F32, name="sx_bc")
    for st_dram, st_ld, st_bc in ((srep, srep_ld, srep_bc), (sx, sx_ld, sx_bc)):
        bcast = bass.AP(
            tensor=st_dram.tensor, offset=st_dram.offset,
            ap=[[0, 128], [st_dram.ap[0][0], HEADS]],
        )
        nc.sync.dma_start(st_ld, bcast)
        # pair-map: column p gets step[2p] on rows 0-63, step[2p+1] on 64-127
        st_ldv = st_ld.rearrange("p (c two) -> p c two", two=2)
        for half in range(2):
            rows = slice(64 * half, 64 * half + 64)
            nc.vector.tensor_copy(st_bc[rows, :], st_ldv[rows, :, half])

    # ---- pass 1 over x: stage+round x to f32r, projT (bf16) ------------
    # rep^T is pooled directly from projT (pooling commutes with W_proj).
    repT_raw = consts.tile([128, NPAIR, NB], F32, name="repT_raw")
    projT = pTp.tile([128, 4, T], BF16, name="projT")
    x_sb = xp.tile([128, 2, T], F32R, name="x_sb")
    for j in range(NCHUNK):
        xc = xp.tile([128, 2, 512], F32, name="xc", tag="xc", bufs=6)
        nc.sync.dma_start(xc, x_r[:, :, ts(j, 512)])
        nc.vector.tensor_copy(x_sb[:, :, ts(j, 512)], xc)
        for g in range(4):
            pt_ps = psum.tile([128, 512], F32, name="pt_ps", tag="mm", bufs=mm_bufs)
            for o in range(2):
                nc.tensor.matmul(
                    pt_ps, wp_r[:, o, ts(g, 128)], x_sb[:, o, ts(j, 512)],
                    start=(o == 0), stop=(o == 1),
                )
            nc.scalar.copy(projT[:, g, ts(j, 512)], pt_ps)
            # block sums: chunk j covers h rows 8j..8j+8 (one block row);
            # local t = hi*64 + wb*8 + wi -> reduce (hi, wi) per block col wb
            nc.vector.reduce_sum(
                repT_raw[:, g, ts(j, 8)],
                projT[:, g, ts(j, 512)].rearrange(
                    "p (hi wb wi) -> p wb hi wi", hi=8, wb=8, wi=8
                ),
                axis=AX.XY,
            )

    # ---- repT [128i, 4, 64blk]; scaled block-diagonal copy for S -------
    # repT_bd[:, p, :] is [[sA, 0], [0, sB]] so one K=128 matmul computes
    # both heads' scores (exact zeros kill the cross-head terms).
    repT = consts.tile([128, NPAIR, NB], F32, name="repT")
    repT_bd = consts.tile([128, NPAIR, 128], BF16, name="repT_bd")
    nc.vector.memset(repT_bd, 0.0)
    for g in range(4):
        nc.vector.tensor_scalar_mul(repT[:, g, :], repT_raw[:, g, :], 1.0 / 64.0)
        for h in range(2):
            rows = slice(64 * h, 64 * h + 64)
            nc.vector.tensor_scalar_mul(
                repT_bd[rows, g, ds(64 * h, 64)], repT_raw[rows, g, :], SCALE / 64.0
            )

    # ---- pass 2 over x: proj [128t, 32, INNER] in bf16 -----------------
    proj_bf = prp.tile([128, NTT, INNER], BF16, name="proj_bf")
    for m in range(NTT):
        pr_ps = psum.tile([128, INNER], F32, name="pr_ps", tag="mm", bufs=mm_bufs)
        for o in range(2):
            nc.tensor.matmul(
                pr_ps, x_sb[:, o, ts(m, 128)], wp_r[:, o, :],
                start=(o == 0), stop=(o == 1),
            )
        if m % 2 == 0:
            nc.vector.tensor_copy(proj_bf[:, m, :], pr_ps)
        else:
            nc.scalar.copy(proj_bf[:, m, :], pr_ps)
    xp.release()

    # ---- attention stages, pipelined per head pair ----------------------
    # P_sb[p]: [128, T]; rows 0-63 = head 2p, rows 64-127 = head 2p+1.
    # Scores are q.k/8 with pooled queries -> bounded |s| ~< 2, so softmax
    # needs no max-subtraction; exp reads each score chunk straight from
    # PSUM with per-chunk partial row sums.
    p_tiles = [
        pp.tile([128, T], BF16, name=f"p{p}", tag=f"p{p}") for p in range(NPAIR)
    ]
    xd2_tiles = []
    for p in range(NPAIR):
        zpart = stats.tile([128, NCHUNK], F32, name="zpart", tag="zpart")
        for j in range(NCHUNK):
            s_ps = psum.tile([128, 512], F32, name="s_ps", tag="mm", bufs=mm_bufs)
            nc.tensor.matmul(
                s_ps, repT_bd[:, p, :], projT[:, p, ts(j, 512)],
                start=True, stop=True,
            )
            nc.scalar.activation(
                out=p_tiles[p][:, ts(j, 512)], in_=s_ps, func=ACTF.Exp,
                bias=0.0, scale=1.0, accum_out=zpart[:, j : j + 1],
            )
        zsum = stats.tile([128, 1], F32, name="zsum", tag="zsum")
        nc.vector.reduce_sum(zsum, zpart, axis=AX.X)
        rz = stats.tile([128, 1], F32, name="rz", tag="rz")
        nc.vector.reciprocal(rz, zsum)
        # P stays unnormalized; 1/Z is folded into rep_delta (via rz_bc) and
        # into the back-projection lhsT (via rz*step_x on xd2).  rz_bc is a
        # [128,128] tile whose every row equals rz^T (rank-1 ones x rz^T).
        rzt_ps = psum.tile([1, 128], F32, name="rzt_ps", tag="b3s", bufs=1)
        nc.tensor.transpose(rzt_ps, rz, ident_f)
        rzt_sb = b3.tile([1, 128], F32, name="rzt_sb", tag="rzt_sb")
        nc.vector.tensor_copy(rzt_sb, rzt_ps)
        rzbc_ps = psum.tile([128, 128], F32, name="rzbc_ps", tag="b3s", bufs=1)
        nc.tensor.matmul(rzbc_ps, ones_row, rzt_sb, start=True, stop=True)
        rz_bc = b3.tile([128, 128], F32, name="rz_bc", tag="rz_bc")
        nc.vector.tensor_copy(rz_bc, rzbc_ps)
        if p == NPAIR - 1:
            pTp.release()
        # rep_delta^T: full-block matmul [128t,128(dA|dB)]^T @ [128t,128(qA|qB)]
        # -> [128 d-pair, 128 q-pair]; diagonal quadrants are the two heads'
        # rep_delta^T (off-diagonal quadrants are unused cross terms).
        av1_ps = psum.tile([128, 128], F32, name="av1_ps", tag="acc64", bufs=1)
        for mq in range(NTT // 4):
            tp_ps = psum.tile([128, 4, 128], BF16, name="tp_ps", tag="tp", bufs=2)
            for c in range(4):
                nc.tensor.transpose(
                    tp_ps[:, c, :], p_tiles[p][:, ts(4 * mq + c, 128)], ident_bf
                )
            ptr_sb = b2.tile([128, 4, 128], BF16, name="ptr_sb", tag="ptr")
            if mq % 2 == 0:
                nc.vector.tensor_copy(ptr_sb, tp_ps)
            else:
                nc.scalar.copy(ptr_sb, tp_ps)
            for c in range(4):
                m = 4 * mq + c
                nc.tensor.matmul(
                    av1_ps, proj_bf[:, m, ds(128 * p, 128)], ptr_sb[:, c, :],
                    start=(m == 0), stop=(m == NTT - 1),
                )
        # rep2^T = repT + step_rep * rep_delta^T   [128 (dA|dB), 64 q]
        rep2T = b3.tile([128, NB], F32, name="rep2T", tag="rep2T")
        for h in range(2):
            pr = slice(64 * h, 64 * h + 64)
            blk = ds(64 * h, 64)
            nc.vector.scalar_tensor_tensor(
                rep2T[pr, :], av1_ps[pr, blk], srep_bc[pr, p : p + 1],
                rz_bc[pr, blk], op0=ALU.mult, op1=ALU.mult,
            )
        nc.vector.tensor_add(rep2T, rep2T, repT[:, p, :])
        rep2T_b = b3.tile([128, NB], BF16, name="rep2T_b", tag="rep2T_b")
        nc.vector.tensor_copy(rep2T_b, rep2T)
        rep2T_bd = b3.tile([128, 128], BF16, name="rep2T_bd", tag="rep2T_bd")
        nc.vector.memset(rep2T_bd, 0.0)
        for h in range(2):
            rows = slice(64 * h, 64 * h + 64)
            nc.vector.tensor_scalar_mul(
                rep2T_bd[rows, ds(64 * h, 64)], rep2T[rows, :], SCALE
            )
        # rep2 (q on partitions): [64 q, 128 (dA|dB)]
        r2_ps = psum.tile([64, 128], BF16, name="r2_ps", tag="b3s", bufs=1)
        nc.tensor.transpose(r2_ps, rep2T_b, ident_bf)
        r2_sb = b3.tile([64, 128], BF16, name="r2_sb", tag="r2_sb")
        nc.vector.tensor_copy(r2_sb, r2_ps)
        # S2 = (scale*rep2) @ rep2.T per head -> [128 (qA|qB), 64 q']
        s2_ps = psum.tile([128, NB], F32, name="s2_ps", tag="b3s", bufs=1)
        nc.tensor.matmul(s2_ps, rep2T_bd, rep2T_b, start=True, stop=True)
        z2 = stats.tile([128, 1], F32, name="z2", tag="z2")
        p2_sb = b3.tile([128, NB], BF16, name="p2_sb", tag="p2_sb")
        nc.scalar.activation(
            out=p2_sb, in_=s2_ps, func=ACTF.Exp,
            bias=0.0, scale=1.0, accum_out=z2,
        )
        rz2 = stats.tile([128, 1], F32, name="rz2", tag="rz2")
        nc.vector.reciprocal(rz2, z2)
        nc.vector.tensor_scalar_mul(p2_sb, p2_sb, rz2)
        # P2^T: [64 q', 128 (qA|qB)]
        p2t_ps = psum.tile([64, 128], BF16, name="p2t_ps", tag="b3s", bufs=1)
        nc.tensor.transpose(p2t_ps, p2_sb, ident_bf)
        p2t_sb = b3.tile([64, 128], BF16, name="p2t_sb", tag="p2t_sb")
        nc.vector.tensor_copy(p2t_sb, p2t_ps)
        # xd2 = P2 @ rep2: one [64,128]x[64,128] -> [128,128] full-block
        # matmul; the diagonal quadrants are the two heads' [q, d] results.
        xd2_ps = psum.tile([128, 128], F32, name="xd2_ps", tag="b3s", bufs=1)
        nc.tensor.matmul(xd2_ps, p2t_sb, r2_sb, start=True, stop=True)
        rzsx = stats.tile([128, 1], F32, name="rzsx", tag="rzsx")
        nc.vector.tensor_mul(rzsx, rz, sx_bc[:, p : p + 1])
        xd2_sb = xd2p.tile([128, DH], BF16, name="xd2_sb", tag=f"xd2_{p}")
        for h in range(2):
            rows = slice(64 * h, 64 * h + 64)
            nc.vector.tensor_scalar_mul(
                xd2_sb[rows, :], xd2_ps[rows, ds(64 * h, 64)],
                rzsx[rows, :],
            )
        xd2_tiles.append(xd2_sb)
    b2.release()
    b3.release()
    prp.release()

    # ---- back-projection + output projection, per 512-token chunk -------
    for j in range(NCHUNK):
        xdt_tiles = []
        for p in range(NPAIR):
            bp_ps = psum.tile([128, 512], F32, name="bp_ps", tag="mm", bufs=mm_bufs)
            for h in range(2):
                pr = slice(64 * h, 64 * h + 64)
                nc.tensor.matmul(
                    bp_ps[pr, :], xd2_tiles[p][pr, :], p_tiles[p][pr, ts(j, 512)],
                    start=True, stop=True, tile_position=(64 * h, 64 * h),
                )
            xdt_sb = xdtp.tile([128, 512], F32R, name="xdt_sb", tag=f"xdt_{p}")
            if p % 2 == 0:
                nc.vector.tensor_copy(xdt_sb, bp_ps)
            else:
                nc.scalar.copy(xdt_sb, bp_ps)
            xdt_tiles.append(xdt_sb)
        for ct in range(2):
            op_ps = psum.tile([128, 512], F32, name="op_ps", tag="mm", bufs=mm_bufs)
            for g in range(4):
                nc.tensor.matmul(
                    op_ps, wo_r[:, g, ts(ct, 128)], xdt_tiles[g],
                    start=(g == 0), stop=(g == 3),
                )
            out_sb = outp.tile([128, 512], F32, name="out_sb", tag="out_sb")
            nc.vector.tensor_tensor(
                out_sb, op_ps, bo_sb[:, ct : ct + 1].to_broadcast((128, 512)),
                ALU.add,
            )
            nc.sync.dma_start(out_r[:, ct, ts(j, 512)], out_sb)
    xd2p.release()
    pp.release()
    xdtp.release()
    outp.release()
    psum.release()
    stats.release()
    consts.release()


_CACHE = {}


class _Runner:
    """Builds the Bass module once and keeps a single jitted shard_map
    executable alive, so repeat kernel() calls skip retracing/relowering."""

    def __init__(self):
        import jax
        import jax.numpy as jnp
        from jax.sharding import Mesh, PartitionSpec
        from jax.experimental.shard_map import shard_map
        from concourse import bass2jax

        self.jax = jax
        nc = build_module()
        self.nc = nc
        bass2jax.install_neuronx_cc_hook()

        partition_name = (
            nc.partition_id_tensor.name if nc.partition_id_tensor else None
        )
        in_names, out_names, out_avals = [], [], []
        for alloc in nc.m.functions[0].allocations:
            if not isinstance(alloc, mybir.MemoryLocationSet):
                continue
            name = alloc.memorylocations[0].name
            if alloc.kind == "ExternalInput":
                if name != partition_name:
                    in_names.append(name)
            elif alloc.kind == "ExternalOutput":
                out_names.append(name)
                out_avals.append(
                    jax.core.ShapedArray(
                        tuple(alloc.tensor_shape), mybir.dt.np(alloc.dtype)
                    )
                )
        n_params = len(in_names)
        n_outs = len(out_avals)
        all_names = list(in_names) + list(out_names)
        if partition_name is not None:
            all_names.append(partition_name)
        self.in_names = in_names
        self.out_names = out_names
        self.out_avals = out_avals

        def _body(*args):
            operands = list(args)
            if partition_name is not None:
                operands.append(bass2jax.partition_id_tensor())
            outs = bass2jax._bass_exec_p.bind(
                *operands,
                out_avals=tuple(out_avals),
                in_names=tuple(all_names),
                out_names=tuple(out_names),
                lowering_input_output_aliases=(),
                sim_require_finite=True,
                sim_require_nnan=True,
                nc=nc,
            )
            return tuple(outs)

        self.body = _body
        devices = jax.devices()[:B]
        mesh = Mesh(np.asarray(devices), ("core",))
        donate = tuple(range(n_params, n_params + n_outs))
        self.sharded = jax.jit(
            shard_map(
                _body, mesh=mesh,
                in_specs=(PartitionSpec("core"),) * (n_params + n_outs),
                out_specs=(PartitionSpec("core"),) * n_outs,
                check_rep=False,
            ),
            donate_argnums=donate,
            keep_unused=True,
        )

    def run(self, in_maps):
        concat_in = [
            np.concatenate([m[name] for m in in_maps], axis=0)
            for name in self.in_names
        ]
        zeros = [
            np.zeros((B * a.shape[0], *a.shape[1:]), a.dtype) for a in self.out_avals
        ]
        out_arrs = self.sharded(*concat_in, *zeros)
        return [
            {
                name: np.asarray(out_arrs[i]).reshape(B, *self.out_avals[i].shape)[c]
                for i, name in enumerate(self.out_names)
            }
            for c in range(B)
        ]

    def bench(self, in_maps, reps=8, inner=72, base=8):
        """Time device-resident executions (no donation, operands staged once).

        Times jitted chains of `base` and `inner` back-to-back kernel
        executions; returns (per_exec_seconds, base_chain_seconds, results)
        with per_exec = (t_inner - t_base) / (inner - base), which amortizes
        away the per-dispatch round-trip of this axon-tunneled environment.
        """
        import time
        from jax.sharding import Mesh, PartitionSpec, NamedSharding
        from jax.experimental.shard_map import shard_map

        jax = self.jax
        devices = jax.devices()[:B]
        mesh = Mesh(np.asarray(devices), ("core",))
        sharding = NamedSharding(mesh, PartitionSpec("core"))
        n_ops = len(self.in_names) + len(self.out_avals)

        def chain(n):
            def f(*args):
                outs = []
                for _ in range(n):
                    outs.extend(self.body(*args))
                return tuple(outs)
            return f

        concat_in = [
            np.concatenate([m[name] for m in in_maps], axis=0)
            for name in self.in_names
        ]
        zeros = [
            np.zeros((B * a.shape[0], *a.shape[1:]), a.dtype) for a in self.out_avals
        ]
        staged = [jax.device_put(a, sharding) for a in concat_in + zeros]

        times = {}
        out1 = None
        for n in (base, inner):
            jfn = jax.jit(
                shard_map(
                    chain(n), mesh=mesh,
                    in_specs=(PartitionSpec("core"),) * n_ops,
                    out_specs=(PartitionSpec("core"),) * (n * len(self.out_avals)),
                    check_rep=False,
                ),
                keep_unused=True,
            )
            out = jfn(*staged)
            jax.block_until_ready(out)
            best = float("inf")
            for _ in range(reps):
                t0 = time.perf_counter()
                out = jfn(*staged)
                jax.block_until_ready(out)
                best = min(best, time.perf_counter() - t0)
            times[n] = best
            if n == base:
                out1 = out
        per_exec = (times[inner] - times[base]) / (inner - base)
        if per_exec <= 0:
            per_exec = times[inner] / inner  # noise floor: report upper bound
        results = [
            {
                name: np.asarray(out1[i]).reshape(B, *self.out_avals[i].shape)[c]
                for i, name in enumerate(self.out_names)
            }
            for c in range(B)
        ]
        return per_exec, times[base], results


def _get_runner():
    key = CFG["p_mode"]
    if key not in _CACHE:
        _CACHE[key] = _Runner()
    return _CACHE[key]


def _make_in_maps(x, W_proj, step_rep, step_x, W_out, b_out):
    x = np.ascontiguousarray(np.asarray(x, dtype=np.float32))
    shared = {
        "w_proj": np.ascontiguousarray(np.asarray(W_proj, dtype=np.float32)),
        "w_out": np.ascontiguousarray(np.asarray(W_out, dtype=np.float32)),
        "b_out": np.ascontiguousarray(np.asarray(b_out, dtype=np.float32)),
        "s_rep": np.ascontiguousarray(
            np.asarray(step_rep, dtype=np.float32).reshape(HEADS)
        ),
        "s_x": np.ascontiguousarray(
            np.asarray(step_x, dtype=np.float32).reshape(HEADS)
        ),
    }
    return [
        {"x": np.ascontiguousarray(x[b].reshape(C, T)), **shared} for b in range(B)
    ]


def kernel(x, W_proj, step_rep, step_x, W_out, b_out):
    runner = _get_runner()
    results = runner.run(_make_in_maps(x, W_proj, step_rep, step_x, W_out, b_out))
    outs = [np.asarray(results[b]["out"]).reshape(C, 64, 64) for b in range(B)]
    return np.stack(outs, axis=0)

